# revision 5
# baseline (speedup 1.0000x reference)
"""Trainium2 Bass kernel for nn_DenoisingNet_MLP_3 (LISTA denoiser, 2 stages).

Strategy: 8 cores = 2 samples x 4 patch-row chunks. The device runs the heavy
per-token pipeline (thr/Wg MLPs, y, 5 LISTA iterations, x_pred) in fp32r
(full-rate FP22 matmuls). The host runs the tiny per-sample ops (sd-MLP/CBAM
-> Dcat/S, ~0.5% of FLOPs), plus unfold slicing and the overlap-add fold.
One compiled NEFF is reused for both stages (weights differ via inputs).
"""
import numpy as np
import concourse.bass as bass
import concourse.bacc as bacc
import concourse.mybir as mybir
import concourse.tile as tile
from concourse.bass_utils import run_bass_kernel_spmd

fp32 = mybir.dt.float32
fp32r = mybir.dt.float32r
Alu = mybir.AluOpType
Act = mybir.ActivationFunctionType

KP = 16            # patch size
P2 = 256           # patch features
DD = 624
PR = 113           # stride-1 patch grid is 113x113
ROWS_PER_CORE = 29
R0S = [0, 28, 56, 84]          # first patch row per core chunk
TILE_ROWS = [4, 4, 4, 4, 4, 3, 3, 3]   # 29 patch rows -> 8 token tiles
LC = ROWS_PER_CORE * PR        # 3277 tokens per core
D_SZ = [128, 128, 128, 128, 112]
T_LISTA = 5

_NC_CACHE: dict = {}


# --------------------------------------------------------------------------
# device program
# --------------------------------------------------------------------------

def _build(c_val: float):
    nc = bacc.Bacc("TRN2", target_bir_lowering=False, debug=False, num_devices=8)

    img = nc.dram_tensor("img44", [44, 128], fp32r, kind="ExternalInput")
    wd = {}
    for pre, dims in (
        ("pd", [(256, 1024), (1024, 512), (512, 512)]),
        ("lam", [(256, 1024), (1024, 512), (512, 112)]),
        ("w", [(256, 1024), (1024, 512), (512, 256)]),
    ):
        for li, (i_, o_) in enumerate(dims, 1):
            wd[f"{pre}{li}w"] = nc.dram_tensor(f"{pre}{li}w", [i_, o_], fp32r, kind="ExternalInput")
            wd[f"{pre}{li}b"] = nc.dram_tensor(f"{pre}{li}b", [o_], fp32, kind="ExternalInput")
    dcat_d = nc.dram_tensor("dcat", [256, DD], fp32r, kind="ExternalInput")
    dcatT_d = nc.dram_tensor("dcatT", [DD, 256], fp32r, kind="ExternalInput")
    smat_d = nc.dram_tensor("smat", [DD, DD], fp32r, kind="ExternalInput")
    i128_d = nc.dram_tensor("i128", [128, 128], fp32r, kind="ExternalInput")
    px_o = nc.dram_tensor("px_o", [256, LC], fp32, kind="ExternalOutput")
    wg_o = nc.dram_tensor("wg_o", [256, LC], fp32, kind="ExternalOutput")

    inv_c = float(1.0 / c_val)

    with tile.TileContext(nc) as tc:
        with (
            tc.tile_pool(name="fx", bufs=1) as fx,      # persistent weights
            tc.tile_pool(name="wk", bufs=1) as wk,      # working tiles (per-tile bufs)
            tc.tile_pool(name="pp", bufs=1, space="PSUM") as pp,
        ):
            # ---- persistent loads ----
            def load_w(name, i_, o_):
                nk = i_ // 128
                t = fx.tile([128, nk * o_], fp32r, name=f"sb_{name}")
                nc.sync.dma_start(
                    out=t[:].rearrange("p (k c) -> p k c", c=o_),
                    in_=bass.AP(wd[name], 0, [[o_, 128], [128 * o_, nk], [1, o_]]),
                )
                return t

            def load_b(name, o_):
                nch = (o_ + 127) // 128
                t = fx.tile([128, nch], fp32, name=f"sb_{name}")
                if o_ % 128 == 0:
                    nc.sync.dma_start(
                        out=t[:], in_=bass.AP(wd[name], 0, [[1, 128], [128, nch]])
                    )
                else:
                    nc.sync.dma_start(out=t[0:o_, 0:1], in_=bass.AP(wd[name], 0, [[1, o_]]))
                return t

            ws = {}
            for pre, dims in (
                ("pd", [(256, 1024), (1024, 512), (512, 512)]),
                ("lam", [(256, 1024), (1024, 512), (512, 112)]),
                ("w", [(256, 1024), (1024, 512), (512, 256)]),
            ):
                for li, (i_, o_) in enumerate(dims, 1):
                    ws[f"{pre}{li}w"] = load_w(f"{pre}{li}w", i_, o_)
                    ws[f"{pre}{li}b"] = load_b(f"{pre}{li}b", o_)

            dcat = fx.tile([128, 2 * DD], fp32r, name="sb_dcat")
            nc.sync.dma_start(
                out=dcat[:].rearrange("p (k c) -> p k c", c=DD),
                in_=bass.AP(dcat_d, 0, [[DD, 128], [128 * DD, 2], [1, DD]]),
            )
            dcatT = fx.tile([128, 5 * 256], fp32r, name="sb_dcatT")
            nc.sync.dma_start(
                out=dcatT[:, 0 : 4 * 256].rearrange("p (k c) -> p k c", c=256),
                in_=bass.AP(dcatT_d, 0, [[256, 128], [128 * 256, 4], [1, 256]]),
            )
            nc.sync.dma_start(
                out=dcatT[0:112, 4 * 256 : 5 * 256],
                in_=bass.AP(dcatT_d, 512 * 256, [[256, 112], [1, 256]]),
            )
            smat = fx.tile([128, 5 * DD], fp32r, name="sb_smat")
            nc.sync.dma_start(
                out=smat[:, 0 : 4 * DD].rearrange("p (k c) -> p k c", c=DD),
                in_=bass.AP(smat_d, 0, [[DD, 128], [128 * DD, 4], [1, DD]]),
            )
            nc.sync.dma_start(
                out=smat[0:112, 4 * DD : 5 * DD],
                in_=bass.AP(smat_d, 512 * DD, [[DD, 112], [1, DD]]),
            )
            i128 = fx.tile([128, 128], fp32r, name="sb_i128")
            nc.sync.dma_start(out=i128[:], in_=i128_d[:])

            # ---- soft-threshold helper: z_out = soft(ps, thr_sl) ----
            def soft(t, tag_sfx, ps, sz, n_, thr_sl, z_out):
                a = wk.tile([128, n_], fp32, name=f"sa{t}_{tag_sfx}", tag="scr_a", bufs=2)
                u = wk.tile([128, n_], fp32, name=f"su{t}_{tag_sfx}", tag="scr_u", bufs=1)
                r = wk.tile([128, n_], fp32, name=f"sr{t}_{tag_sfx}", tag="scr_r", bufs=2)
                tt = wk.tile([128, n_], fp32, name=f"st{t}_{tag_sfx}", tag="scr_t", bufs=1)
                # soft(x, l) = sign(x) * (max(|x|, l) - l)   [= sign(x)*relu(|x|-l)]
                nc.scalar.activation(a[0:sz], ps[0:sz], Act.Abs)
                nc.vector.tensor_tensor(u[0:sz], a[0:sz], thr_sl, Alu.max)
                nc.vector.tensor_tensor(r[0:sz], u[0:sz], thr_sl, Alu.subtract)
                nc.scalar.activation(tt[0:sz], ps[0:sz], Act.Sign)
                nc.vector.tensor_tensor(z_out, tt[0:sz], r[0:sz], Alu.mult)

            # ---- token-tile loop ----
            rstart = 0
            for t, rows in enumerate(TILE_ROWS):
                Nv = PR * rows          # valid tokens
                N = Nv + (Nv % 2)       # fp32r matmul needs even moving dim
                tok0 = PR * rstart

                uf = [
                    wk.tile([128, N], fp32r, name=f"uf{t}_{c}", tag=f"uf{c}", bufs=2)
                    for c in range(2)
                ]
                for c in range(2):
                    for kh8 in range(8):
                        kh = 8 * c + kh8
                        nc.sync.dma_start(
                            out=uf[c][16 * kh8 : 16 * kh8 + 16, 0:Nv].rearrange(
                                "kw (r j) -> kw r j", j=PR
                            ),
                            in_=bass.AP(img, (rstart + kh) * 128, [[1, 16], [128, rows], [1, PR]]),
                        )
                    if N > Nv:
                        # pad column: duplicate token (rstart, j=0)
                        nc.sync.dma_start(
                            out=uf[c][:, Nv:N],
                            in_=bass.AP(img, (rstart + 8 * c) * 128, [[128, 8], [1, 16]]),
                        )

                thr = wk.tile([128, 5 * N], fp32, name=f"thr{t}", tag="thr", bufs=1)
                wg = wk.tile([128, 2 * N], fp32, name=f"wg{t}", tag="wg", bufs=2)

                # -- MLP over this tile: L1(256->1024) L2(1024->512) L3(512->nout) --
                def emit_mlp(pre, nout3, out_cb):
                    w1t, b1t = ws[f"{pre}1w"], ws[f"{pre}1b"]
                    w2t, b2t = ws[f"{pre}2w"], ws[f"{pre}2b"]
                    w3t = ws[f"{pre}3w"]
                    ps2 = [
                        pp.tile([128, N], fp32, name=f"ps2_{t}_{pre}_{m}", tag="ps", bufs=8)
                        for m in range(4)
                    ]
                    for half in (0, 1):
                        h1a = wk.tile([128, 4 * N], fp32r, name=f"h1_{t}_{pre}_{half}", tag="h1a", bufs=2)
                        for i in range(4):
                            kg = half * 4 + i
                            ps1 = pp.tile([128, N], fp32, name=f"ps1_{t}_{pre}_{kg}", tag="ps", bufs=8)
                            nc.tensor.matmul(
                                ps1[:], w1t[:, kg * 128 : kg * 128 + 128], uf[0][:],
                                start=True, stop=False,
                            )
                            nc.tensor.matmul(
                                ps1[:], w1t[:, 1024 + kg * 128 : 1024 + kg * 128 + 128], uf[1][:],
                                start=False, stop=True,
                            )
                            nc.scalar.activation(
                                h1a[:, i * N : (i + 1) * N], ps1[:], Act.Relu,
                                bias=b1t[:, kg : kg + 1],
                            )
                        for m in range(4):
                            for i in range(4):
                                kg = half * 4 + i
                                nc.tensor.matmul(
                                    ps2[m][:],
                                    w2t[:, kg * 512 + m * 128 : kg * 512 + m * 128 + 128],
                                    h1a[:, i * N : (i + 1) * N],
                                    start=(half == 0 and i == 0),
                                    stop=(half == 1 and i == 3),
                                )
                    h2 = wk.tile([128, 4 * N], fp32r, name=f"h2_{t}_{pre}", tag="h2", bufs=1)
                    for m in range(4):
                        nc.scalar.activation(
                            h2[:, m * N : (m + 1) * N], ps2[m][:], Act.Relu,
                            bias=b2t[:, m : m + 1],
                        )
                    for mo in range((nout3 + 127) // 128):
                        sz = min(128, nout3 - mo * 128)
                        ps3 = pp.tile([128, N], fp32, name=f"ps3_{t}_{pre}_{mo}", tag="ps", bufs=8)
                        for k in range(4):
                            nc.tensor.matmul(
                                ps3[0:sz],
                                w3t[:, k * nout3 + mo * 128 : k * nout3 + mo * 128 + sz],
                                h2[:, k * N : (k + 1) * N],
                                start=(k == 0), stop=(k == 3),
                            )
                        out_cb(mo, sz, ps3)

                def pd_out(mo, sz, ps3):
                    nc.scalar.activation(
                        thr[:, mo * N : (mo + 1) * N], ps3[:], Act.Identity,
                        bias=ws["pd3b"][:, mo : mo + 1],
                    )

                def lam_out(mo, sz, ps3):
                    nc.scalar.activation(
                        thr[0:112, 4 * N : 5 * N], ps3[0:112], Act.Identity,
                        bias=ws["lam3b"][0:112, 0:1],
                    )

                def w_out(mo, sz, ps3):
                    nc.scalar.activation(
                        wg[:, mo * N : (mo + 1) * N], ps3[:], Act.Sigmoid,
                        bias=ws["w3b"][:, mo : mo + 1],
                    )

                emit_mlp("pd", 512, pd_out)
                emit_mlp("lam", 112, lam_out)
                emit_mlp("w", 256, w_out)

                # -- y = Dcat^T uf (feature-major), yc = y/c, z0 = soft(y, thr) --
                yc = wk.tile([128, 5 * N], fp32r, name=f"yc{t}", tag="yc", bufs=1)
                z = wk.tile([128, 5 * N], fp32r, name=f"z{t}_0", tag="z", bufs=2)
                for mc in range(5):
                    sz = D_SZ[mc]
                    d0 = 128 * mc
                    psy = pp.tile([128, N], fp32, name=f"psy_{t}_{mc}", tag="ps", bufs=8)
                    nc.tensor.matmul(
                        psy[0:sz], dcat[:, d0 : d0 + sz], uf[0][:], start=True, stop=False
                    )
                    nc.tensor.matmul(
                        psy[0:sz], dcat[:, DD + d0 : DD + d0 + sz], uf[1][:],
                        start=False, stop=True,
                    )
                    nc.vector.tensor_scalar(
                        yc[0:sz, mc * N : (mc + 1) * N], psy[0:sz], inv_c, None, Alu.mult
                    )
                    soft(t, f"z0_{mc}", psy, sz, N,
                         thr[0:sz, mc * N : (mc + 1) * N],
                         z[0:sz, mc * N : (mc + 1) * N])

                # -- LISTA iterations: z = soft(z @ S + y/c, thr) --
                for it in range(T_LISTA):
                    zn = wk.tile([128, 5 * N], fp32r, name=f"z{t}_{it + 1}", tag="z", bufs=2)
                    for mc in range(5):
                        sz = D_SZ[mc]
                        d0 = 128 * mc
                        psl = pp.tile([128, N], fp32, name=f"psl_{t}_{it}_{mc}", tag="ps", bufs=8)
                        nc.tensor.matmul(
                            psl[0:sz], i128[0:sz, 0:sz], yc[0:sz, mc * N : (mc + 1) * N],
                            start=True, stop=False,
                        )
                        for kc in range(5):
                            szk = D_SZ[kc]
                            nc.tensor.matmul(
                                psl[0:sz],
                                smat[0:szk, kc * DD + d0 : kc * DD + d0 + sz],
                                z[0:szk, kc * N : (kc + 1) * N],
                                start=False, stop=(kc == 4),
                            )
                        soft(t, f"i{it}_{mc}", psl, sz, N,
                             thr[0:sz, mc * N : (mc + 1) * N],
                             zn[0:sz, mc * N : (mc + 1) * N])
                    z = zn

                # -- x_pred = clip(z @ DcatT, 0, 1) * Wg ; outputs --
                for fc in range(2):
                    psx = pp.tile([128, N], fp32, name=f"psx_{t}_{fc}", tag="ps", bufs=8)
                    for kc in range(5):
                        szk = D_SZ[kc]
                        nc.tensor.matmul(
                            psx[:],
                            dcatT[0:szk, kc * 256 + fc * 128 : kc * 256 + fc * 128 + 128],
                            z[0:szk, kc * N : (kc + 1) * N],
                            start=(kc == 0), stop=(kc == 4),
                        )
                    clp = wk.tile([128, N], fp32, name=f"clp{t}_{fc}", tag="clp", bufs=2)
                    nc.vector.tensor_scalar(clp[:], psx[:], 0.0, 1.0, Alu.max, Alu.min)
                    px = wk.tile([128, N], fp32, name=f"px{t}_{fc}", tag="px", bufs=2)
                    nc.vector.tensor_tensor(px[:], clp[:], wg[:, fc * N : (fc + 1) * N], Alu.mult)
                    nc.sync.dma_start(
                        out=bass.AP(px_o, fc * 128 * LC + tok0, [[LC, 128], [1, Nv]]),
                        in_=px[:, 0:Nv],
                    )
                    nc.sync.dma_start(
                        out=bass.AP(wg_o, fc * 128 * LC + tok0, [[LC, 128], [1, Nv]]),
                        in_=wg[:, fc * N : fc * N + Nv],
                    )

                rstart += rows

    nc.compile()
    return nc


# --------------------------------------------------------------------------
# host-side small ops (per sample): ext -> sd MLP -> CBAM -> Dcat/S
# --------------------------------------------------------------------------

def _host_sd(img2d, p, c_val):
    # ext: stride-8 unfold, every 2nd patch, first 112   [112, 256]
    ext = np.empty((112, 256), np.float32)
    for tt in range(112):
        ir, ic = divmod(2 * tt, 15)
        ext[tt] = img2d[8 * ir : 8 * ir + 16, 8 * ic : 8 * ic + 16].reshape(256)
    h = ext
    for wname, bname in (("s1w", "s1b"), ("s2w", "s2b"), ("s3w", "s3b")):
        h = np.maximum(h @ p[wname] + p[bname], 0.0, dtype=np.float32)
    sd = (h @ p["s4w"] + p["s4b"]).astype(np.float32)          # [112, 256]
    nrm = np.maximum(np.linalg.norm(sd, axis=-1, keepdims=True), 1e-12)
    sd = (sd / nrm).astype(np.float32)
    v = sd.T.reshape(256, 8, 14)                                # channels, 8x14
    # channel attention
    def camlp(vec):
        return np.maximum(vec @ p["caw1"], 0.0) @ p["caw2"]
    ca = 1.0 / (1.0 + np.exp(-(camlp(v.mean(axis=(1, 2))) + camlp(v.max(axis=(1, 2))))))
    v = (v * ca[:, None, None]).astype(np.float32)
    # spatial attention: 7x7 conv on [mean_c, max_c], pad 3
    s2 = np.stack([v.mean(axis=0), v.max(axis=0)])              # [2, 8, 14]
    pad = np.zeros((2, 14, 20), np.float32)
    pad[:, 3:11, 3:17] = s2
    sa = np.zeros((8, 14), np.float32)
    saw = p["saw"][0]                                           # [2,7,7]
    for ch in range(2):
        for dy in range(7):
            for dx in range(7):
                sa += saw[ch, dy, dx] * pad[ch, dy : dy + 8, dx : dx + 14]
    v = (v * (1.0 / (1.0 + np.exp(-sa)))[None]).astype(np.float32)
    sd = v.reshape(256, 112)
    dcat = np.concatenate([p["Dict"], sd], axis=1).astype(np.float32)   # [256, 624]
    smat = (np.eye(DD, dtype=np.float32) - (dcat.T @ dcat) / c_val).astype(np.float32)
    return dcat, np.ascontiguousarray(dcat.T), smat


def _fold(pfull):
    # pfull [256, 12769] feature-major -> overlap-add [128,128]
    out = np.zeros((128, 128), np.float32)
    pr = pfull.reshape(16, 16, PR, PR)
    for kh in range(16):
        for kw in range(16):
            out[kh : kh + PR, kw : kw + PR] += pr[kh, kw]
    return out


def _assemble(chunks):
    # chunks: list of 4 arrays [256, 3277] -> [256, 12769]
    full = np.empty((256, PR * PR), np.float32)
    for g in range(PR):
        q = 0 if g <= 28 else (g - 1) // 28
        loc = g - 28 * q
        full[:, g * PR : (g + 1) * PR] = chunks[q][:, loc * PR : (loc + 1) * PR]
    return full


# --------------------------------------------------------------------------
# stage driver
# --------------------------------------------------------------------------

def _run_stage(nc, imgs, p, lam_pre, pd_pre, c_val, results_holder=None, trace=False):
    per_sample = []
    for n in range(2):
        per_sample.append(_host_sd(imgs[n], p, c_val))

    base = {"i128": np.eye(128, dtype=np.float32)}
    for li, src in ((1, f"{pd_pre}1"), (2, f"{pd_pre}2"), (3, f"{pd_pre}3")):
        w = p[src + "w"]
        b = p[src + "b"]
        if li == 3:
            w = (w / c_val).astype(np.float32)
            b = (b / c_val).astype(np.float32)
        base[f"pd{li}w"] = np.ascontiguousarray(w)
        base[f"pd{li}b"] = np.ascontiguousarray(b)
    for li, src in ((1, f"{lam_pre}1"), (2, f"{lam_pre}2"), (3, f"{lam_pre}3")):
        w = p[src + "w"]
        b = p[src + "b"]
        if li == 3:
            w = (w / c_val).astype(np.float32)
            b = (b / c_val).astype(np.float32)
        base[f"lam{li}w"] = np.ascontiguousarray(w)
        base[f"lam{li}b"] = np.ascontiguousarray(b)
    for li in (1, 2, 3):
        base[f"w{li}w"] = np.ascontiguousarray(p[f"w{li}w"])
        base[f"w{li}b"] = np.ascontiguousarray(p[f"w{li}b"])

    in_maps = []
    for core in range(8):
        n, q = divmod(core, 4)
        dcat, dcatT, smat = per_sample[n]
        m = dict(base)
        m["img44"] = np.ascontiguousarray(imgs[n][R0S[q] : R0S[q] + 44, :])
        m["dcat"] = dcat
        m["dcatT"] = dcatT
        m["smat"] = smat
        in_maps.append(m)

    res = run_bass_kernel_spmd(nc, in_maps, list(range(8)), trace=trace)
    if results_holder is not None:
        results_holder.append(res)

    out = np.empty((2, 128, 128), np.float32)
    for n in range(2):
        px = _assemble([res.results[4 * n + q]["px_o"] for q in range(4)])
        wgf = _assemble([res.results[4 * n + q]["wg_o"] for q in range(4)])
        num = _fold(px)
        den = _fold(wgf)
        out[n] = num / den
    return out


def kernel(**inputs) -> np.ndarray:
    p = {k: np.asarray(v, np.float32) for k, v in inputs.items()}
    c_val = float(np.asarray(inputs["c"]))
    key = ("nc", c_val)
    if key not in _NC_CACHE:
        _NC_CACHE[key] = _build(c_val)
    nc = _NC_CACHE[key]
    x = p["x"]  # [2,1,128,128]
    imgs1 = [x[n, 0] for n in range(2)]
    res1 = _run_stage(nc, imgs1, p, "a", "p", c_val)
    imgs2 = [res1[n] for n in range(2)]
    res2 = _run_stage(nc, imgs2, p, "b", "q", c_val)
    return res2.reshape(2, 1, 128, 128).astype(np.float32)
